# revision 14
# baseline (speedup 1.0000x reference)
"""Multi-head attention (B=4, L=2048, D=768, H=12) on 8 Trainium2 NeuronCores.

Sharding: (batch, head-group). Core c handles batch c//2 and heads
6*(c%2) .. 6*(c%2)+6.  Each core computes its 6 heads' attention output and
the partial output projection y_part = AO @ Wo[rows]; the host sums the two
partials per batch, rescales, and adds biases.  No collectives.

Per-core pipeline (fp16 operands; fp32 PSUM accumulate):
  A. QKV projections in fp8e4m3 DoubleRow with an exact 3-term residual
     decomposition (all terms at x256 scale, one PSUM group each):
        256*x@W = x8@W256 + xr16@W16 + x8@Wr
     where x8=fp8(x), xr16=fp8(16(x-x8)), W256=fp8(256W), W16=fp8(16W),
     Wr=fp8(256W - W256).  Eviction converts to fp16 (Q adds 256*bq; bk is
     dropped -- softmax shift-invariant; bv/bo folded on the host).
  B. Scores in fp16: S^T[k,q] = K_h @ Q_h^T -> PSUM pairs [128, 2, 512]
     (two key-chunks per exp instruction); p = exp(s/8 - 2.5) in fp16,
     computed on ACT (real exp, ~60% of units) and via Schraudolph on DVE
     (uint16 bits ARE the fp16 pattern; saturation at 0 handles underflow).
     PV runs query-major: pv[q, qc, 0:64] accumulated over all 16 key
     chunks of a query-half; the denominator accumulates via ones-matmuls
     into a separate PSUM bank.  AO = 256*attn in fp16.
  C. Two query-half sweeps (qc 0-7 then 8-15) over all heads, so the AO
     transpose (XBAR) + output projection + y DMA of the first half overlap
     the second sweep.  y is fp16 at 256x; host: (y0+y1)/256 + bv@Wo + bo.
  D. ~3.4us of dummy matmuls at kernel start burn through the PE p-state
     ramp while the x DMAs land, so real matmuls run at full clock.
"""

import numpy as np
import ml_dtypes

import concourse.mybir as mybir
import concourse.tile as tile
from concourse import bacc
from concourse.bass_utils import run_bass_kernel_spmd

F32 = mybir.dt.float32
F16 = mybir.dt.float16
U16 = mybir.dt.uint16
FP8 = mybir.dt.float8e4
P = 128
B, L, D, H = 4, 2048, 768, 12
HD = 64                    # head dim
HL = H // 2                # heads per core = 6
HO = HL * HD               # local feature dim = 384
KC = D // P                # contraction chunks over D = 6
CP = KC // 2               # DoubleRow contraction chunk-pairs = 3
LC = L // P                # key chunks = 16
MC = HO // P               # feature chunks = 3
QC = LC                    # query chunks = 16
DR = mybir.MatmulPerfMode.DoubleRow

LOG2E = 1.4426950408889634
SHIFT = 2.5
# scores psum = (256Q)*(256K) = 65536 * s;  p = exp(s/8 - SHIFT)
ACT_SCALE = 0.125 / 65536.0
SCH_A = 1024.0 * LOG2E / (8.0 * 65536.0)
SCH_C = -60.0
SCH_B = 15 * 1024 - SHIFT * LOG2E * 1024.0 + SCH_C

PV_LAG = 4                 # PV trails scores by this many pair-units
N_WARMUP = 8               # dummy matmuls burning the PE p-state ramp

# exp engine pattern: ~60% ACT (real exp), 40% DVE (Schraudolph)
EXP_PAT = (0, 1, 0, 1, 0, 0, 1, 0, 1, 0)  # 6 ACT : 4 DVE per 10

_NC = None


def s512(i):
    return slice(i * 512, (i + 1) * 512)


def build():
    nc = bacc.Bacc("TRN2", target_bir_lowering=False, debug=False)

    # x blocks of 512 queries, contiguous per (partition, block)
    x8 = nc.dram_tensor("x8", [P, 4, CP, 2, 512], FP8, kind="ExternalInput")
    xr = nc.dram_tensor("xr", [P, 4, CP, 2, 512], FP8, kind="ExternalInput")
    w_names = []
    for t in ("q", "k", "v"):
        for v in ("a", "b", "r"):       # a=W256, b=W16, r=Wr
            w_names.append(f"w{t}{v}")
    w_dram = {
        n: nc.dram_tensor(n, [P, CP * 2 * HO], FP8, kind="ExternalInput")
        for n in w_names
    }
    wo = nc.dram_tensor("wo", [P, MC * D], F16, kind="ExternalInput")
    bq = nc.dram_tensor("bq", [HO], F32, kind="ExternalInput")
    y = nc.dram_tensor("y", [L, D], F16, kind="ExternalOutput")

    with tile.TileContext(nc) as tc:
        with tc.tile_pool(name="static", bufs=1) as static:
            qT_tiles = [static.tile([P, L], F16, name=f"qT{m}") for m in range(MC)]
            kT_tiles = [static.tile([P, L], F16, name=f"kT{m}") for m in range(MC)]
            v_sb = static.tile([P, LC, HL, HD], F16)
            ones_sb = static.tile([P, 1], F16)
            ao_q = static.tile([P, QC, HL, HD], F16)      # query-major AO
            ao_t = static.tile([P, QC, MC, P], F16)       # feature-major AO
            bq_sb = static.tile([P, MC], F32)
            shift_sb = static.tile([P, 1], F32)
            dummy_sb = static.tile([P, 512], F16)
            dummy_ps_out = static.tile([P, 1], F32)

            nc.vector.memset(ones_sb[:], 0.0625)          # dn = sum(p)/16
            nc.vector.memset(shift_sb[:], -SHIFT)
            nc.vector.memset(dummy_sb[:], 0.0)
            # preload the exp activation table while DMAs run
            nc.scalar.activation(
                out=dummy_ps_out[:],
                in_=shift_sb[:],
                func=mybir.ActivationFunctionType.Exp,
            )

            with (
                tc.tile_pool(name="xpool", bufs=1) as xpool,
                tc.tile_pool(name="wpool", bufs=1) as wpool,
            ):
                x8_sb = xpool.tile([P, 4, CP, 2, 512], FP8, name="x8")
                xr_sb = xpool.tile([P, 4, CP, 2, 512], FP8, name="xr")
                w_sb = {
                    n: wpool.tile([P, CP, 2, HO], FP8, name=n) for n in w_names
                }
                wo_sb = wpool.tile([P, MC, D], F16, name="wo")

                def w_dma(n):
                    nc.sync.dma_start(
                        w_sb[n][:],
                        w_dram[n].ap().rearrange(
                            "p (c t h) -> p c t h", c=CP, t=2
                        ),
                    )

                # DMA order: the first projection jobs (wq*, block 0) first.
                w_dma("wqa")
                nc.sync.dma_start(x8_sb[:, 0], x8[:, 0])
                w_dma("wqb")
                nc.sync.dma_start(xr_sb[:, 0], xr[:, 0])
                w_dma("wqr")
                for v in ("a", "b", "r"):
                    w_dma(f"wk{v}")
                for j in range(1, 4):
                    nc.sync.dma_start(x8_sb[:, j], x8[:, j])
                    nc.sync.dma_start(xr_sb[:, j], xr[:, j])
                for v in ("a", "b", "r"):
                    w_dma(f"wv{v}")
                nc.sync.dma_start(bq_sb[:], bq.ap().rearrange("(c p) -> p c", p=P))
                nc.sync.dma_start(
                    wo_sb[:, :, :], wo.ap().rearrange("p (c d) -> p c d", c=MC)
                )

                with (
                    tc.tile_pool(name="ppool", bufs=10) as ppool,
                    tc.tile_pool(name="rpool", bufs=4) as rpool,
                    tc.tile_pool(name="ypool", bufs=4) as ypool,
                    tc.tile_pool(name="sps", bufs=3, space="PSUM") as sps,
                    tc.tile_pool(name="pvps", bufs=1, space="PSUM") as pvps,
                    tc.tile_pool(name="dnps", bufs=1, space="PSUM") as dnps,
                ):
                    # ---- PE p-state warm-up: one chained accumulation of
                    # dummy matmuls on zeros (back-to-back, no sem gaps) ----
                    wps = sps.tile([P, 2, 512], F32, tag="s", name="wu")
                    for i in range(N_WARMUP):
                        nc.tensor.matmul(
                            wps[:, 0, :],
                            dummy_sb[:, 0:P],
                            dummy_sb[:, :],
                            start=(i == 0),
                            stop=(i == N_WARMUP - 1),
                        )

                    dn_t = dnps.tile([P, 2, 8], F32, name="dn")
                    nc.vector.memset(dn_t[:, :, :], 0.0)

                    # engine-balance accumulators (ns) for evictions
                    eng_load = {"A": 0.0, "D": 0.0}

                    def pick_engine(act_cost, dve_cost):
                        if eng_load["A"] + act_cost <= eng_load["D"] + dve_cost:
                            eng_load["A"] += act_cost
                            return "A"
                        eng_load["D"] += dve_cost
                        return "D"

                    def emit_proj(w3, m, j, out_cb):
                        """3-term fp8 DR projection of one [128, 512] slice:
                        features m*128.., queries j*512..; out_cb(psum)."""
                        ps = sps.tile([P, 2, 512], F32, tag="s", name=f"pj{m}_{j}")
                        wa, wb, wr = w3
                        msl = slice(m * P, (m + 1) * P)
                        first = True
                        for wt, xt in ((wa, x8_sb), (wb, xr_sb), (wr, x8_sb)):
                            for cp in range(CP):
                                nc.tensor.matmul(
                                    ps[:, 0, :],
                                    wt[:, cp, :, msl],
                                    xt[:, j, cp, :, :],
                                    start=first,
                                    stop=(wt is wr and cp == CP - 1),
                                    perf_mode=DR,
                                    skip_group_check=True,
                                )
                                first = False
                        out_cb(ps)

                    def do_qk(which, m, j):
                        w3 = tuple(w_sb[f"w{which}{v}"] for v in ("a", "b", "r"))

                        def evict(ps):
                            out_ap = (qT_tiles if which == "q" else kT_tiles)[m][
                                :, s512(j)
                            ]
                            e = pick_engine(570.0, 658.0)
                            if which == "q":
                                if e == "A":
                                    nc.scalar.activation(
                                        out=out_ap,
                                        in_=ps[:, 0, :],
                                        func=mybir.ActivationFunctionType.Identity,
                                        bias=bq_sb[:, m : m + 1],
                                    )
                                else:
                                    nc.vector.tensor_scalar(
                                        out_ap,
                                        ps[:, 0, :],
                                        1.0,
                                        bq_sb[:, m : m + 1],
                                        mybir.AluOpType.mult,
                                        mybir.AluOpType.add,
                                    )
                            else:
                                if e == "A":
                                    nc.scalar.activation(
                                        out=out_ap,
                                        in_=ps[:, 0, :],
                                        func=mybir.ActivationFunctionType.Copy,
                                    )
                                else:
                                    nc.vector.tensor_copy(out_ap, ps[:, 0, :])

                        emit_proj(w3, m, j, evict)

                    def do_v(l):
                        """V projection for key-chunk l -> v_sb[:, l, :, :]."""
                        ps = sps.tile([P, 2, 512], F32, tag="s", name=f"vj{l}")
                        wa, wb, wr = (w_sb[f"wv{v}"] for v in ("a", "b", "r"))
                        lsl = slice((l % 4) * P, (l % 4 + 1) * P)
                        first = True
                        for wt, xt in ((wa, x8_sb), (wb, xr_sb), (wr, x8_sb)):
                            for cp in range(CP):
                                nc.tensor.matmul(
                                    ps[:, 0, 0:HO],
                                    xt[:, l // 4, cp, :, lsl],
                                    wt[:, cp, :, :],
                                    start=first,
                                    stop=(wt is wr and cp == CP - 1),
                                    perf_mode=DR,
                                    skip_group_check=True,
                                )
                                first = False
                        e = pick_engine(463.0, 525.0)
                        dst = v_sb[:, l, :, :]
                        src = ps[:, 0, 0:HO].rearrange("p (h d) -> p h d", d=HD)
                        if e == "A":
                            nc.scalar.activation(
                                out=dst, in_=src,
                                func=mybir.ActivationFunctionType.Copy,
                            )
                        else:
                            nc.vector.tensor_copy(dst, src)

                    exp_idx = [0]

                    def emit_scores_exp(hl, lp, q4):
                        """Two scores matmuls (lk=2lp, 2lp+1) + one 1024-wide
                        exp into a fp16 P tile [P, 2, 512]."""
                        pc, odd = hl // 2, hl % 2
                        r0 = odd * HD
                        s_t = sps.tile([P, 2, 512], F32, tag="s",
                                       name=f"s{hl}_{lp}_{q4}")
                        for t in range(2):
                            lk = 2 * lp + t
                            nc.tensor.matmul(
                                s_t[:, t, :],
                                kT_tiles[pc][r0 : r0 + HD, lk * P : (lk + 1) * P],
                                qT_tiles[pc][r0 : r0 + HD, s512(q4)],
                                start=True,
                                stop=True,
                            )
                        p_t = ppool.tile([P, 2, 512], F16, tag="p",
                                         name=f"p{hl}_{lp}_{q4}")
                        use_act = EXP_PAT[exp_idx[0] % len(EXP_PAT)] == 0
                        exp_idx[0] += 1
                        s_flat = s_t[:].rearrange("p a b -> p (a b)")
                        p_flat = p_t[:].rearrange("p a b -> p (a b)")
                        if use_act:
                            nc.scalar.activation(
                                out=p_flat,
                                in_=s_flat,
                                func=mybir.ActivationFunctionType.Exp,
                                bias=shift_sb[:, 0:1],
                                scale=ACT_SCALE,
                            )
                            eng_load["A"] += 996.0
                        else:
                            nc.vector.tensor_scalar(
                                p_flat.bitcast(U16),
                                s_flat,
                                SCH_A,
                                SCH_B,
                                mybir.AluOpType.mult,
                                mybir.AluOpType.add,
                            )
                            eng_load["D"] += 1192.0
                        return p_t

                    def emit_pv(pv, hl, lp, q4, p_t, sweep):
                        first = lp == 0 and q4 % 2 == 0
                        last = lp == LC // 2 - 1
                        for t in range(2):
                            for jj in range(4):
                                qc8 = (q4 % 2) * 4 + jj
                                nc.tensor.matmul(
                                    pv[:, qc8, :],
                                    p_t[:, t, jj * P : (jj + 1) * P],
                                    v_sb[:, 2 * lp + t, hl, :],
                                    start=(first and t == 0 and jj == 0),
                                    stop=(last and t == 1),
                                    skip_group_check=True,
                                )
                                nc.tensor.matmul(
                                    dn_t[:, hl % 2, qc8 : qc8 + 1],
                                    p_t[:, t, jj * P : (jj + 1) * P],
                                    ones_sb[:, :],
                                    start=False,
                                    stop=(last and t == 1),
                                    skip_group_check=True,
                                )

                    def evict_ao(pv, hl, sweep):
                        """recip + one batched scaled eviction of 8 qc's."""
                        rstage = rpool.tile([P, 8], F32, tag="r")
                        nc.vector.reciprocal(rstage[:, :], dn_t[:, hl % 2, :])
                        eng_load["D"] += 135.0
                        if hl + 2 < HL or sweep == 0:
                            nc.vector.memset(dn_t[:, hl % 2, :], 0.0)
                        qc0 = sweep * 8
                        rb = rstage[:, :, None].broadcast_to((P, 8, HD))
                        nc.vector.scalar_tensor_tensor(
                            ao_q[:, qc0 : qc0 + 8, hl, :],
                            pv[:, :, :],
                            0.0625,
                            rb,
                            mybir.AluOpType.mult,
                            mybir.AluOpType.mult,
                        )
                        eng_load["D"] += 658.0

                    def quad_transpose(g):
                        nc.sync.dma_start_transpose(
                            ao_t[:, 4 * g : 4 * g + 4, :, :],
                            ao_q[:, 4 * g : 4 * g + 4, :, :],
                        )

                    def do_outproj(m):
                        ps = sps.tile([P, 2, 512], F32, tag="s", name=f"y{m}")
                        yp = ps[:].rearrange("p a b -> p (a b)")
                        for c in range(MC):
                            for n0, nsz in ((0, 512), (512, 256)):
                                nc.tensor.matmul(
                                    yp[:, n0 : n0 + nsz],
                                    ao_t[:, m, c, :],
                                    wo_sb[:, c, n0 : n0 + nsz],
                                    start=(c == 0),
                                    stop=(c == MC - 1),
                                )
                        y_t = ypool.tile([P, D], F16, tag="yt")
                        e = pick_engine(783.0, 925.0)
                        if e == "A":
                            nc.scalar.activation(
                                out=y_t[:], in_=yp[:, 0:D],
                                func=mybir.ActivationFunctionType.Copy,
                            )
                        else:
                            nc.vector.tensor_copy(y_t[:], yp[:, 0:D])
                        yeng = nc.sync if m % 2 == 0 else nc.scalar
                        yeng.dma_start(y[m * P : (m + 1) * P, :], y_t[:])

                    # ---------------- prelude ----------------
                    # qT/kT chunk 0 (heads 0-1) fully, then V chunks 0-1.
                    for j in range(4):
                        do_qk("q", 0, j)
                        do_qk("k", 0, j)
                    do_v(0)
                    do_v(1)

                    # jobs woven into the unit stream:
                    #   unit u (0..191): sweep u//96, head (u%96)//16
                    prejobs = {}
                    for j in range(4):                     # qT/kT chunk 1
                        prejobs.setdefault(2 * j, []).append(
                            lambda j=j: do_qk("q", 1, j))
                        prejobs.setdefault(2 * j + 1, []).append(
                            lambda j=j: do_qk("k", 1, j))
                    for l in range(2, LC):                 # V chunks 2-15
                        prejobs.setdefault(l - 2, []).append(lambda l=l: do_v(l))
                    for j in range(4):                     # qT/kT chunk 2
                        prejobs.setdefault(16 + 2 * j, []).append(
                            lambda j=j: do_qk("q", 2, j))
                        prejobs.setdefault(16 + 2 * j + 1, []).append(
                            lambda j=j: do_qk("k", 2, j))
                    # sweep B: transposes of qc 0-7, then outproj jobs
                    prejobs.setdefault(98, []).append(lambda: quad_transpose(0))
                    prejobs.setdefault(100, []).append(lambda: quad_transpose(1))
                    for i in range(8):                     # outproj qc 0-7
                        prejobs.setdefault(104 + 8 * i, []).append(
                            lambda i=i: do_outproj(i))

                    units = [
                        (sweep, hl, lp, q4h)
                        for sweep in range(2)
                        for hl in range(HL)
                        for lp in range(LC // 2)
                        for q4h in range(2)
                    ]
                    pend = []             # (hl, lp, q4, p_t, sweep)
                    pv_cur = [None]
                    prev_key = [None]

                    for u, (sweep, hl, lp, q4h) in enumerate(units):
                        q4 = 2 * sweep + q4h
                        for job in prejobs.get(u, ()):
                            job()
                        if lp == 0 and q4h == 0:
                            if pv_cur[0] is not None:
                                while pend:
                                    emit_pv(pv_cur[0], *pend.pop(0))
                                evict_ao(pv_cur[0], *prev_key[0])
                            pv_cur[0] = pvps.tile([P, 8, HD], F32, tag="pv",
                                                  name=f"pv{sweep}_{hl}")
                            prev_key[0] = (hl, sweep)
                        p_t = emit_scores_exp(hl, lp, q4)
                        pend.append((hl, lp, q4, p_t, sweep))
                        while len(pend) > PV_LAG:
                            emit_pv(pv_cur[0], *pend.pop(0))
                    while pend:
                        emit_pv(pv_cur[0], *pend.pop(0))
                    evict_ao(pv_cur[0], HL - 1, 1)

                    # ---------------- tail: qc 8-15 ----------------
                    quad_transpose(2)
                    quad_transpose(3)
                    for i in range(8, QC):
                        do_outproj(i)

    nc.compile()
    return nc


def _get_nc():
    global _NC
    if _NC is None:
        _NC = build()
    return _NC


E4NP = ml_dtypes.float8_e4m3
F16NP = ml_dtypes.float16 if hasattr(ml_dtypes, "float16") else np.float16


def _dr_rows_x(a):
    """[768, 2048] -> [128, 4, 3, 2, 512]: query-block-major DR layout;
    row (cp, t, p) holds input row cp*256 + t*128 + p."""
    return np.ascontiguousarray(
        a.reshape(CP, 2, P, 4, 512).transpose(2, 3, 0, 1, 4)
    )


def _w_tensors(W):
    """W [768, 384] fp32 -> (W256, W16, Wr) fp8, flattened [128, 3*2*384]
    with row (cp, t, p) holding input row cp*256 + t*128 + p."""
    Wa = (256.0 * W).astype(E4NP)
    Wb = (16.0 * W).astype(E4NP)
    Wr = (256.0 * W - Wa.astype(np.float32)).astype(E4NP)
    return tuple(
        np.ascontiguousarray(
            t.reshape(CP, 2, P, HO).transpose(2, 0, 1, 3).reshape(P, CP * 2 * HO)
        )
        for t in (Wa, Wb, Wr)
    )


def kernel(**inputs) -> np.ndarray:
    x = np.asarray(inputs["x"], dtype=np.float32)
    Wq = np.asarray(inputs["Wq"], dtype=np.float32)
    Wk = np.asarray(inputs["Wk"], dtype=np.float32)
    Wv = np.asarray(inputs["Wv"], dtype=np.float32)
    Wo = np.asarray(inputs["Wo"], dtype=np.float32)
    bq = np.asarray(inputs["bq"], dtype=np.float32)
    bv = np.asarray(inputs["bv"], dtype=np.float32)
    bo = np.asarray(inputs["bo"], dtype=np.float32)

    nc = _get_nc()

    in_maps = []
    for c in range(8):
        b, hg = c // 2, c % 2
        cs = slice(hg * HO, (hg + 1) * HO)
        xT = np.ascontiguousarray(x[b].T)               # [768, 2048]
        x8f = xT.astype(E4NP)
        xrf = (16.0 * (xT - x8f.astype(np.float32))).astype(E4NP)
        m = {"x8": _dr_rows_x(x8f), "xr": _dr_rows_x(xrf)}
        for t, W in (("q", Wq), ("k", Wk), ("v", Wv)):
            Ws = W[:, cs]
            for v, arr in zip(("a", "b", "r"), _w_tensors(Ws)):
                m[f"w{t}{v}"] = arr
        Wos = Wo[cs, :]                                  # [384, 768]
        m["wo"] = np.ascontiguousarray(
            Wos.reshape(MC, P, D).transpose(1, 0, 2).reshape(P, MC * D)
        ).astype(np.float16)
        m["bq"] = np.ascontiguousarray(256.0 * bq[cs])
        in_maps.append(m)

    res = run_bass_kernel_spmd(nc, in_maps, core_ids=list(range(8)))
    bias_full = bv @ Wo + bo
    out = np.empty((B, L, D), dtype=np.float32)
    for b in range(B):
        out[b] = (
            res.results[2 * b]["y"].astype(np.float32)
            + res.results[2 * b + 1]["y"].astype(np.float32)
        ) / 256.0 + bias_full
    return out


# revision 15
# speedup vs baseline: 1.0135x; 1.0135x over previous
"""Multi-head attention (B=4, L=2048, D=768, H=12) on 8 Trainium2 NeuronCores.

Sharding: (batch, head-group). Core c handles batch c//2 and heads
6*(c%2) .. 6*(c%2)+6.  Each core computes its 6 heads' attention output and
the partial output projection y_part = AO @ Wo[rows]; the host sums the two
partials per batch, rescales, and adds biases.  No collectives.

Per-core pipeline (fp16 operands; fp32 PSUM accumulate):
  A. QKV projections in fp8e4m3 DoubleRow with an exact 3-term residual
     decomposition (all terms at x256 scale, one PSUM group each):
        256*x@W = x8@W256 + xr16@W16 + x8@Wr
     where x8=fp8(x), xr16=fp8(16(x-x8)), W256=fp8(256W), W16=fp8(16W),
     Wr=fp8(256W - W256).  Eviction converts to fp16 (Q adds 256*bq; bk is
     dropped -- softmax shift-invariant; bv/bo folded on the host).
  B. Scores in fp16: S^T[k,q] = K_h @ Q_h^T -> PSUM pairs [128, 2, 512]
     (two key-chunks per exp instruction); p = exp(s/8 - 2.5) in fp16,
     computed on ACT (real exp, ~60% of units) and via Schraudolph on DVE
     (uint16 bits ARE the fp16 pattern; saturation at 0 handles underflow).
     PV runs query-major: pv[q, qc, 0:64] accumulated over all 16 key
     chunks of a query-half; the denominator accumulates via ones-matmuls
     into a separate PSUM bank.  AO = 256*attn in fp16.
  C. Two query-half sweeps (qc 0-7 then 8-15) over all heads, so the AO
     transpose (XBAR) + output projection + y DMA of the first half overlap
     the second sweep.  y is fp16 at 256x; host: (y0+y1)/256 + bv@Wo + bo.
  D. ~3.4us of dummy matmuls at kernel start burn through the PE p-state
     ramp while the x DMAs land, so real matmuls run at full clock.
"""

import numpy as np
import ml_dtypes

import concourse.mybir as mybir
import concourse.tile as tile
from concourse import bacc
from concourse.bass_utils import run_bass_kernel_spmd

F32 = mybir.dt.float32
F16 = mybir.dt.float16
U16 = mybir.dt.uint16
FP8 = mybir.dt.float8e4
P = 128
B, L, D, H = 4, 2048, 768, 12
HD = 64                    # head dim
HL = H // 2                # heads per core = 6
HO = HL * HD               # local feature dim = 384
KC = D // P                # contraction chunks over D = 6
CP = KC // 2               # DoubleRow contraction chunk-pairs = 3
LC = L // P                # key chunks = 16
MC = HO // P               # feature chunks = 3
QC = LC                    # query chunks = 16
DR = mybir.MatmulPerfMode.DoubleRow

LOG2E = 1.4426950408889634
SHIFT = 2.5
# scores psum = (256Q)*(256K) = 65536 * s;  p = exp(s/8 - SHIFT)
ACT_SCALE = 0.125 / 65536.0
SCH_A = 1024.0 * LOG2E / (8.0 * 65536.0)
SCH_C = -60.0
SCH_B = 15 * 1024 - SHIFT * LOG2E * 1024.0 + SCH_C

PV_LAG = 4                 # PV trails scores by this many pair-units
N_WARMUP = 8               # dummy matmuls burning the PE p-state ramp

# exp engine pattern: ~60% ACT (real exp), 40% DVE (Schraudolph)
EXP_PAT = (0, 1, 0, 1, 0, 0, 1, 0, 1, 0)  # 6 ACT : 4 DVE per 10

_NC = None


def s512(i):
    return slice(i * 512, (i + 1) * 512)


def build():
    nc = bacc.Bacc("TRN2", target_bir_lowering=False, debug=False)

    # x blocks of 512 queries, contiguous per (partition, block)
    x8 = nc.dram_tensor("x8", [P, 4, CP, 2, 512], FP8, kind="ExternalInput")
    xr = nc.dram_tensor("xr", [P, 4, CP, 2, 512], FP8, kind="ExternalInput")
    w_names = []
    for t in ("q", "k", "v"):
        for v in ("a", "b", "r"):       # a=W256, b=W16, r=Wr
            w_names.append(f"w{t}{v}")
    w_dram = {
        n: nc.dram_tensor(n, [P, CP * 2 * HO], FP8, kind="ExternalInput")
        for n in w_names
    }
    wo = nc.dram_tensor("wo", [P, MC * D], F16, kind="ExternalInput")
    bq = nc.dram_tensor("bq", [HO], F32, kind="ExternalInput")
    y = nc.dram_tensor("y", [L, D], F16, kind="ExternalOutput")

    with tile.TileContext(nc) as tc:
        with tc.tile_pool(name="static", bufs=1) as static:
            qT_tiles = [static.tile([P, L], F16, name=f"qT{m}") for m in range(MC)]
            kT_tiles = [static.tile([P, L], F16, name=f"kT{m}") for m in range(MC)]
            v_sb = static.tile([P, LC, HL, HD], F16)
            ones_sb = static.tile([P, 1], F16)
            ao_q = static.tile([P, QC, HL, HD], F16)      # query-major AO
            ao_t = static.tile([P, QC, MC, P], F16)       # feature-major AO
            bq_sb = static.tile([P, MC], F32)
            shift_sb = static.tile([P, 1], F32)
            dummy_sb = static.tile([P, 512], F16)
            dummy_ps_out = static.tile([P, 1], F32)

            nc.vector.memset(ones_sb[:], 0.0625)          # dn = sum(p)/16
            nc.vector.memset(shift_sb[:], -SHIFT)
            nc.vector.memset(dummy_sb[:], 0.0)
            # preload the exp activation table while DMAs run
            nc.scalar.activation(
                out=dummy_ps_out[:],
                in_=shift_sb[:],
                func=mybir.ActivationFunctionType.Exp,
            )

            with (
                tc.tile_pool(name="xpool", bufs=1) as xpool,
                tc.tile_pool(name="wpool", bufs=1) as wpool,
            ):
                x8_sb = xpool.tile([P, 4, CP, 2, 512], FP8, name="x8")
                xr_sb = xpool.tile([P, 4, CP, 2, 512], FP8, name="xr")
                w_sb = {
                    n: wpool.tile([P, CP, 2, HO], FP8, name=n) for n in w_names
                }
                wo_sb = wpool.tile([P, MC, D], F16, name="wo")

                def w_dma(n):
                    nc.sync.dma_start(
                        w_sb[n][:],
                        w_dram[n].ap().rearrange(
                            "p (c t h) -> p c t h", c=CP, t=2
                        ),
                    )

                # DMA order: the first projection jobs (wq*, block 0) first.
                w_dma("wqa")
                nc.sync.dma_start(x8_sb[:, 0], x8[:, 0])
                w_dma("wqb")
                nc.sync.dma_start(xr_sb[:, 0], xr[:, 0])
                w_dma("wqr")
                for v in ("a", "b", "r"):
                    w_dma(f"wk{v}")
                for j in range(1, 4):
                    nc.sync.dma_start(x8_sb[:, j], x8[:, j])
                    nc.sync.dma_start(xr_sb[:, j], xr[:, j])
                for v in ("a", "b", "r"):
                    w_dma(f"wv{v}")
                nc.sync.dma_start(bq_sb[:], bq.ap().rearrange("(c p) -> p c", p=P))
                nc.sync.dma_start(
                    wo_sb[:, :, :], wo.ap().rearrange("p (c d) -> p c d", c=MC)
                )

                with (
                    tc.tile_pool(name="ppool", bufs=10) as ppool,
                    tc.tile_pool(name="rpool", bufs=4) as rpool,
                    tc.tile_pool(name="ypool", bufs=4) as ypool,
                    tc.tile_pool(name="sps", bufs=3, space="PSUM") as sps,
                    tc.tile_pool(name="pvps", bufs=1, space="PSUM") as pvps,
                    tc.tile_pool(name="dnps", bufs=1, space="PSUM") as dnps,
                ):
                    # ---- PE p-state warm-up: one chained accumulation of
                    # dummy matmuls on zeros (back-to-back, no sem gaps) ----
                    wps = sps.tile([P, 2, 512], F32, tag="s", name="wu")
                    for i in range(N_WARMUP):
                        nc.tensor.matmul(
                            wps[:, 0, :],
                            dummy_sb[:, 0:P],
                            dummy_sb[:, :],
                            start=(i == 0),
                            stop=(i == N_WARMUP - 1),
                        )

                    dn_t = dnps.tile([P, 2, 8], F32, name="dn")
                    nc.vector.memset(dn_t[:, :, :], 0.0)

                    # engine-balance accumulators (ns) for evictions
                    eng_load = {"A": 0.0, "D": 0.0}

                    def pick_engine(act_cost, dve_cost):
                        if eng_load["A"] + act_cost <= eng_load["D"] + dve_cost:
                            eng_load["A"] += act_cost
                            return "A"
                        eng_load["D"] += dve_cost
                        return "D"

                    def emit_proj(w3, m, j, out_cb):
                        """3-term fp8 DR projection of one [128, 512] slice:
                        features m*128.., queries j*512..; out_cb(psum)."""
                        ps = sps.tile([P, 2, 512], F32, tag="s", name=f"pj{m}_{j}")
                        wa, wb, wr = w3
                        msl = slice(m * P, (m + 1) * P)
                        first = True
                        for wt, xt in ((wa, x8_sb), (wb, xr_sb), (wr, x8_sb)):
                            for cp in range(CP):
                                nc.tensor.matmul(
                                    ps[:, 0, :],
                                    wt[:, cp, :, msl],
                                    xt[:, j, cp, :, :],
                                    start=first,
                                    stop=(wt is wr and cp == CP - 1),
                                    perf_mode=DR,
                                    skip_group_check=True,
                                )
                                first = False
                        out_cb(ps)

                    def do_qk(which, m, j):
                        w3 = tuple(w_sb[f"w{which}{v}"] for v in ("a", "b", "r"))

                        def evict(ps):
                            out_ap = (qT_tiles if which == "q" else kT_tiles)[m][
                                :, s512(j)
                            ]
                            e = pick_engine(570.0, 658.0)
                            if which == "q":
                                if e == "A":
                                    nc.scalar.activation(
                                        out=out_ap,
                                        in_=ps[:, 0, :],
                                        func=mybir.ActivationFunctionType.Identity,
                                        bias=bq_sb[:, m : m + 1],
                                    )
                                else:
                                    nc.vector.tensor_scalar(
                                        out_ap,
                                        ps[:, 0, :],
                                        1.0,
                                        bq_sb[:, m : m + 1],
                                        mybir.AluOpType.mult,
                                        mybir.AluOpType.add,
                                    )
                            else:
                                if e == "A":
                                    nc.scalar.activation(
                                        out=out_ap,
                                        in_=ps[:, 0, :],
                                        func=mybir.ActivationFunctionType.Copy,
                                    )
                                else:
                                    nc.vector.tensor_copy(out_ap, ps[:, 0, :])

                        emit_proj(w3, m, j, evict)

                    def do_v(l):
                        """V projection for key-chunk l -> v_sb[:, l, :, :]."""
                        ps = sps.tile([P, 2, 512], F32, tag="s", name=f"vj{l}")
                        wa, wb, wr = (w_sb[f"wv{v}"] for v in ("a", "b", "r"))
                        lsl = slice((l % 4) * P, (l % 4 + 1) * P)
                        first = True
                        for wt, xt in ((wa, x8_sb), (wb, xr_sb), (wr, x8_sb)):
                            for cp in range(CP):
                                nc.tensor.matmul(
                                    ps[:, 0, 0:HO],
                                    xt[:, l // 4, cp, :, lsl],
                                    wt[:, cp, :, :],
                                    start=first,
                                    stop=(wt is wr and cp == CP - 1),
                                    perf_mode=DR,
                                    skip_group_check=True,
                                )
                                first = False
                        e = pick_engine(463.0, 525.0)
                        dst = v_sb[:, l, :, :]
                        src = ps[:, 0, 0:HO].rearrange("p (h d) -> p h d", d=HD)
                        if e == "A":
                            nc.scalar.activation(
                                out=dst, in_=src,
                                func=mybir.ActivationFunctionType.Copy,
                            )
                        else:
                            nc.vector.tensor_copy(dst, src)

                    exp_idx = [0]

                    def emit_scores_exp(hl, lp, q4):
                        """Two scores matmuls (lk=2lp, 2lp+1) + one 1024-wide
                        exp into a fp16 P tile [P, 2, 512]."""
                        pc, odd = hl // 2, hl % 2
                        r0 = odd * HD
                        s_t = sps.tile([P, 2, 512], F32, tag="s",
                                       name=f"s{hl}_{lp}_{q4}")
                        for t in range(2):
                            lk = 2 * lp + t
                            nc.tensor.matmul(
                                s_t[:, t, :],
                                kT_tiles[pc][r0 : r0 + HD, lk * P : (lk + 1) * P],
                                qT_tiles[pc][r0 : r0 + HD, s512(q4)],
                                start=True,
                                stop=True,
                            )
                        p_t = ppool.tile([P, 2, 512], F16, tag="p",
                                         name=f"p{hl}_{lp}_{q4}")
                        use_act = EXP_PAT[exp_idx[0] % len(EXP_PAT)] == 0
                        exp_idx[0] += 1
                        s_flat = s_t[:].rearrange("p a b -> p (a b)")
                        p_flat = p_t[:].rearrange("p a b -> p (a b)")
                        if use_act:
                            nc.scalar.activation(
                                out=p_flat,
                                in_=s_flat,
                                func=mybir.ActivationFunctionType.Exp,
                                bias=shift_sb[:, 0:1],
                                scale=ACT_SCALE,
                            )
                            eng_load["A"] += 996.0
                        else:
                            nc.vector.tensor_scalar(
                                p_flat.bitcast(U16),
                                s_flat,
                                SCH_A,
                                SCH_B,
                                mybir.AluOpType.mult,
                                mybir.AluOpType.add,
                            )
                            eng_load["D"] += 1192.0
                        return p_t

                    def emit_pv(pv, hl, lp, q4, p_t, sweep):
                        first = lp == 0 and q4 % 2 == 0
                        last = lp == LC // 2 - 1
                        for t in range(2):
                            for jj in range(4):
                                qc8 = (q4 % 2) * 4 + jj
                                nc.tensor.matmul(
                                    pv[:, qc8, :],
                                    p_t[:, t, jj * P : (jj + 1) * P],
                                    v_sb[:, 2 * lp + t, hl, :],
                                    start=(first and t == 0 and jj == 0),
                                    stop=(last and t == 1),
                                    skip_group_check=True,
                                )
                                nc.tensor.matmul(
                                    dn_t[:, hl % 2, qc8 : qc8 + 1],
                                    p_t[:, t, jj * P : (jj + 1) * P],
                                    ones_sb[:, :],
                                    start=False,
                                    stop=(last and t == 1),
                                    skip_group_check=True,
                                )

                    def evict_ao(pv, hl, sweep):
                        """recip + one batched scaled eviction of 8 qc's."""
                        rstage = rpool.tile([P, 8], F32, tag="r")
                        nc.vector.reciprocal(rstage[:, :], dn_t[:, hl % 2, :])
                        eng_load["D"] += 135.0
                        if hl + 2 < HL or sweep == 0:
                            nc.vector.memset(dn_t[:, hl % 2, :], 0.0)
                        qc0 = sweep * 8
                        rb = rstage[:, :, None].broadcast_to((P, 8, HD))
                        nc.vector.scalar_tensor_tensor(
                            ao_q[:, qc0 : qc0 + 8, hl, :],
                            pv[:, :, :],
                            0.0625,
                            rb,
                            mybir.AluOpType.mult,
                            mybir.AluOpType.mult,
                        )
                        eng_load["D"] += 658.0

                    def quad_transpose(g):
                        nc.sync.dma_start_transpose(
                            ao_t[:, 4 * g : 4 * g + 4, :, :],
                            ao_q[:, 4 * g : 4 * g + 4, :, :],
                        )

                    def do_outproj(m):
                        ps = sps.tile([P, 2, 512], F32, tag="s", name=f"y{m}")
                        yp = ps[:].rearrange("p a b -> p (a b)")
                        for c in range(MC):
                            for n0, nsz in ((0, 512), (512, 256)):
                                nc.tensor.matmul(
                                    yp[:, n0 : n0 + nsz],
                                    ao_t[:, m, c, :],
                                    wo_sb[:, c, n0 : n0 + nsz],
                                    start=(c == 0),
                                    stop=(c == MC - 1),
                                )
                        y_t = ypool.tile([P, D], F16, tag="yt")
                        e = pick_engine(783.0, 925.0)
                        if e == "A":
                            nc.scalar.activation(
                                out=y_t[:], in_=yp[:, 0:D],
                                func=mybir.ActivationFunctionType.Copy,
                            )
                        else:
                            nc.vector.tensor_copy(y_t[:], yp[:, 0:D])
                        yeng = nc.sync if m % 2 == 0 else nc.scalar
                        yeng.dma_start(y[m * P : (m + 1) * P, :], y_t[:])

                    # ---------------- prelude ----------------
                    # qT/kT chunk 0 (heads 0-1) fully, then V chunks 0-1.
                    for j in range(4):
                        do_qk("q", 0, j)
                        do_qk("k", 0, j)
                    do_v(0)
                    do_v(1)

                    # jobs woven into the unit stream:
                    #   unit u (0..191): sweep u//96, head (u%96)//16
                    prejobs = {}
                    for j in range(4):                     # qT/kT chunk 1
                        prejobs.setdefault(2 * j, []).append(
                            lambda j=j: do_qk("q", 1, j))
                        prejobs.setdefault(2 * j + 1, []).append(
                            lambda j=j: do_qk("k", 1, j))
                    for l in range(2, LC):                 # V chunks 2-15
                        prejobs.setdefault(l - 2, []).append(lambda l=l: do_v(l))
                    for j in range(4):                     # qT/kT chunk 2
                        prejobs.setdefault(16 + 2 * j, []).append(
                            lambda j=j: do_qk("q", 2, j))
                        prejobs.setdefault(16 + 2 * j + 1, []).append(
                            lambda j=j: do_qk("k", 2, j))
                    # sweep B: transposes of qc 0-7, then outproj jobs
                    prejobs.setdefault(101, []).append(lambda: quad_transpose(0))
                    prejobs.setdefault(103, []).append(lambda: quad_transpose(1))
                    for i in range(8):                     # outproj qc 0-7
                        prejobs.setdefault(105 + 8 * i, []).append(
                            lambda i=i: do_outproj(i))

                    units = [
                        (sweep, hl, lp, q4h)
                        for sweep in range(2)
                        for hl in range(HL)
                        for lp in range(LC // 2)
                        for q4h in range(2)
                    ]
                    pend = []             # (hl, lp, q4, p_t, sweep)
                    pv_state = {"tile": None, "key": None}

                    def pump_pv(entry):
                        """Emit one PV unit; on head-change evict the old pv
                        accumulator first, then (re)allocate the single-bank
                        pv tile.  All accesses to the old tile happen before
                        the new allocation, as bufs=1 requires."""
                        hl, lp, q4, p_t, sweep = entry
                        key = (sweep, hl)
                        if pv_state["key"] != key:
                            if pv_state["tile"] is not None:
                                ohl, osweep = (pv_state["key"][1],
                                               pv_state["key"][0])
                                evict_ao(pv_state["tile"], ohl, osweep)
                            pv_state["tile"] = pvps.tile(
                                [P, 8, HD], F32, tag="pv",
                                name=f"pv{sweep}_{hl}")
                            pv_state["key"] = key
                        emit_pv(pv_state["tile"], hl, lp, q4, p_t, sweep)

                    for u, (sweep, hl, lp, q4h) in enumerate(units):
                        q4 = 2 * sweep + q4h
                        for job in prejobs.get(u, ()):
                            job()
                        p_t = emit_scores_exp(hl, lp, q4)
                        pend.append((hl, lp, q4, p_t, sweep))
                        while len(pend) > PV_LAG:
                            pump_pv(pend.pop(0))
                    while pend:
                        pump_pv(pend.pop(0))
                    evict_ao(pv_state["tile"], HL - 1, 1)

                    # ---------------- tail: qc 8-15 ----------------
                    quad_transpose(2)
                    quad_transpose(3)
                    for i in range(8, QC):
                        do_outproj(i)

    nc.compile()
    return nc


def _get_nc():
    global _NC
    if _NC is None:
        _NC = build()
    return _NC


E4NP = ml_dtypes.float8_e4m3
F16NP = ml_dtypes.float16 if hasattr(ml_dtypes, "float16") else np.float16


def _dr_rows_x(a):
    """[768, 2048] -> [128, 4, 3, 2, 512]: query-block-major DR layout;
    row (cp, t, p) holds input row cp*256 + t*128 + p."""
    return np.ascontiguousarray(
        a.reshape(CP, 2, P, 4, 512).transpose(2, 3, 0, 1, 4)
    )


def _w_tensors(W):
    """W [768, 384] fp32 -> (W256, W16, Wr) fp8, flattened [128, 3*2*384]
    with row (cp, t, p) holding input row cp*256 + t*128 + p."""
    Wa = (256.0 * W).astype(E4NP)
    Wb = (16.0 * W).astype(E4NP)
    Wr = (256.0 * W - Wa.astype(np.float32)).astype(E4NP)
    return tuple(
        np.ascontiguousarray(
            t.reshape(CP, 2, P, HO).transpose(2, 0, 1, 3).reshape(P, CP * 2 * HO)
        )
        for t in (Wa, Wb, Wr)
    )


def kernel(**inputs) -> np.ndarray:
    x = np.asarray(inputs["x"], dtype=np.float32)
    Wq = np.asarray(inputs["Wq"], dtype=np.float32)
    Wk = np.asarray(inputs["Wk"], dtype=np.float32)
    Wv = np.asarray(inputs["Wv"], dtype=np.float32)
    Wo = np.asarray(inputs["Wo"], dtype=np.float32)
    bq = np.asarray(inputs["bq"], dtype=np.float32)
    bv = np.asarray(inputs["bv"], dtype=np.float32)
    bo = np.asarray(inputs["bo"], dtype=np.float32)

    nc = _get_nc()

    in_maps = []
    for c in range(8):
        b, hg = c // 2, c % 2
        cs = slice(hg * HO, (hg + 1) * HO)
        xT = np.ascontiguousarray(x[b].T)               # [768, 2048]
        x8f = xT.astype(E4NP)
        xrf = (16.0 * (xT - x8f.astype(np.float32))).astype(E4NP)
        m = {"x8": _dr_rows_x(x8f), "xr": _dr_rows_x(xrf)}
        for t, W in (("q", Wq), ("k", Wk), ("v", Wv)):
            Ws = W[:, cs]
            for v, arr in zip(("a", "b", "r"), _w_tensors(Ws)):
                m[f"w{t}{v}"] = arr
        Wos = Wo[cs, :]                                  # [384, 768]
        m["wo"] = np.ascontiguousarray(
            Wos.reshape(MC, P, D).transpose(1, 0, 2).reshape(P, MC * D)
        ).astype(np.float16)
        m["bq"] = np.ascontiguousarray(256.0 * bq[cs])
        in_maps.append(m)

    res = run_bass_kernel_spmd(nc, in_maps, core_ids=list(range(8)))
    bias_full = bv @ Wo + bo
    out = np.empty((B, L, D), dtype=np.float32)
    for b in range(B):
        out[b] = (
            res.results[2 * b]["y"].astype(np.float32)
            + res.results[2 * b + 1]["y"].astype(np.float32)
        ) / 256.0 + bias_full
    return out


# revision 17
# speedup vs baseline: 1.0768x; 1.0625x over previous
"""Multi-head attention (B=4, L=2048, D=768, H=12) on 8 Trainium2 NeuronCores.

Sharding: (batch, head-group). Core c handles batch c//2 and heads
6*(c%2) .. 6*(c%2)+6.  Each core computes its 6 heads' attention output and
the partial output projection y_part = AO @ Wo[rows]; the host sums the two
partials per batch, rescales, and adds biases.  No collectives.

Per-core pipeline (fp16 operands; fp32 PSUM accumulate):
  A. QKV projections in fp8e4m3 DoubleRow with an exact 3-term residual
     decomposition (all terms at x256 scale, one PSUM group each):
        256*x@W = x8@W256 + xr16@W16 + x8@Wr
     where x8=fp8(x), xr16=fp8(16(x-x8)), W256=fp8(256W), W16=fp8(16W),
     Wr=fp8(256W - W256).  Eviction converts to fp16 (Q adds 256*bq; bk is
     dropped -- softmax shift-invariant; bv/bo folded on the host).
  B. Scores in fp16: S^T[k,q] = K_h @ Q_h^T -> PSUM [128, 512];
     p = exp(s/8 - 2.5) in fp16, computed on ACT (real exp) or DVE
     (Schraudolph: the uint16 bits ARE the fp16 pattern; uint16 saturation
     at 0 handles underflow), balanced by running engine-load accumulators.
     PV runs query-major: pv[q, qc8, 0:64] accumulated over the 16 key
     chunks of one query-half; denominators via ones-matmuls in their own
     PSUM bank.  AO = 256*attn in fp16.
  C. Two query-half sweeps (qc 0-7 then 8-15) over all heads, so the AO
     transpose (XBAR) + output projection + y DMA of the first half overlap
     the second sweep.  y is fp16 at 256x; host: (y0+y1)/256 + bv@Wo + bo.
  D. A long run of tiny dummy matmuls at kernel start keeps PE busy through
     the p-state ramp until the first input DMAs land, so all real matmuls
     run at full clock (PE idle gaps reset the ramp).
"""

import numpy as np
import ml_dtypes

import concourse.mybir as mybir
import concourse.tile as tile
from concourse import bacc
from concourse.bass_utils import run_bass_kernel_spmd

F32 = mybir.dt.float32
F16 = mybir.dt.float16
U16 = mybir.dt.uint16
FP8 = mybir.dt.float8e4
P = 128
B, L, D, H = 4, 2048, 768, 12
HD = 64                    # head dim
HL = H // 2                # heads per core = 6
HO = HL * HD               # local feature dim = 384
KC = D // P                # contraction chunks over D = 6
CP = KC // 2               # DoubleRow contraction chunk-pairs = 3
LC = L // P                # key chunks = 16
MC = HO // P               # feature chunks = 3
QC = LC                    # query chunks = 16
DR = mybir.MatmulPerfMode.DoubleRow

LOG2E = 1.4426950408889634
SHIFT = 2.5
# scores psum = (256Q)*(256K) = 65536 * s;  p = exp(s/8 - SHIFT)
ACT_SCALE = 0.125 / 65536.0
SCH_A = 1024.0 * LOG2E / (8.0 * 65536.0)
SCH_C = -60.0
SCH_B = 15 * 1024 - SHIFT * LOG2E * 1024.0 + SCH_C

PV_LAG = 6                 # PV trails scores by this many lk-units
N_WARMUP = 160             # tiny dummy matmuls bridging the p-state ramp

# engine time models (ns) for load balancing
ACT_EXP = 570.0
DVE_EXP = 658.0

_NC = None


def s512(i):
    return slice(i * 512, (i + 1) * 512)


def build():
    nc = bacc.Bacc("TRN2", target_bir_lowering=False, debug=False)

    # x blocks of 512 queries, contiguous per (partition, block)
    x8 = nc.dram_tensor("x8", [P, 4, CP, 2, 512], FP8, kind="ExternalInput")
    xr = nc.dram_tensor("xr", [P, 4, CP, 2, 512], FP8, kind="ExternalInput")
    w_names = []
    for t in ("q", "k", "v"):
        for v in ("a", "b", "r"):       # a=W256, b=W16, r=Wr
            w_names.append(f"w{t}{v}")
    w_dram = {
        n: nc.dram_tensor(n, [P, CP * 2 * HO], FP8, kind="ExternalInput")
        for n in w_names
    }
    wo = nc.dram_tensor("wo", [P, MC * D], F16, kind="ExternalInput")
    bq = nc.dram_tensor("bq", [HO], F32, kind="ExternalInput")
    y = nc.dram_tensor("y", [L, D], F16, kind="ExternalOutput")

    with tile.TileContext(nc) as tc:
        with tc.tile_pool(name="static", bufs=1) as static:
            qT_tiles = [static.tile([P, L], F16, name=f"qT{m}") for m in range(MC)]
            kT_tiles = [static.tile([P, L], F16, name=f"kT{m}") for m in range(MC)]
            v_sb = static.tile([P, LC, HL, HD], F16)
            ones_sb = static.tile([P, 1], F16)
            ao_q = static.tile([P, QC, HL, HD], F16)      # query-major AO
            ao_t = static.tile([P, QC, MC, P], F16)       # feature-major AO
            bq_sb = static.tile([P, MC], F32)
            shift_sb = static.tile([P, 1], F32)
            dummy_sb = static.tile([P, 64], F16)
            dummy_ps_out = static.tile([P, 1], F32)

            nc.vector.memset(ones_sb[:], 0.0625)          # dn = sum(p)/16
            nc.vector.memset(shift_sb[:], -SHIFT)
            nc.vector.memset(dummy_sb[:], 0.0)
            # preload the exp activation table while DMAs run
            nc.scalar.activation(
                out=dummy_ps_out[:],
                in_=shift_sb[:],
                func=mybir.ActivationFunctionType.Exp,
            )

            with (
                tc.tile_pool(name="xpool", bufs=1) as xpool,
                tc.tile_pool(name="wpool", bufs=1) as wpool,
            ):
                x8_sb = xpool.tile([P, 4, CP, 2, 512], FP8, name="x8")
                xr_sb = xpool.tile([P, 4, CP, 2, 512], FP8, name="xr")
                w_sb = {
                    n: wpool.tile([P, CP, 2, HO], FP8, name=n) for n in w_names
                }
                wo_sb = wpool.tile([P, MC, D], F16, name="wo")

                def w_dma(n):
                    nc.sync.dma_start(
                        w_sb[n][:],
                        w_dram[n].ap().rearrange(
                            "p (c t h) -> p c t h", c=CP, t=2
                        ),
                    )

                # DMA order matches prelude consumption:
                #   Q00 K00 Q01 K01 V0 V1 Q02 K02 Q03 K03
                w_dma("wqa")
                nc.sync.dma_start(x8_sb[:, 0], x8[:, 0])
                nc.sync.dma_start(xr_sb[:, 0], xr[:, 0])
                w_dma("wqb")
                w_dma("wqr")
                for v in ("a", "b", "r"):
                    w_dma(f"wk{v}")
                nc.sync.dma_start(x8_sb[:, 1], x8[:, 1])
                nc.sync.dma_start(xr_sb[:, 1], xr[:, 1])
                for v in ("a", "b", "r"):
                    w_dma(f"wv{v}")
                for j in (2, 3):
                    nc.sync.dma_start(x8_sb[:, j], x8[:, j])
                    nc.sync.dma_start(xr_sb[:, j], xr[:, j])
                nc.sync.dma_start(bq_sb[:], bq.ap().rearrange("(c p) -> p c", p=P))
                nc.sync.dma_start(
                    wo_sb[:, :, :], wo.ap().rearrange("p (c d) -> p c d", c=MC)
                )

                with (
                    tc.tile_pool(name="ppool", bufs=12) as ppool,
                    tc.tile_pool(name="rpool", bufs=4) as rpool,
                    tc.tile_pool(name="ypool", bufs=4) as ypool,
                    tc.tile_pool(name="sps", bufs=6, space="PSUM") as sps,
                    tc.tile_pool(name="pvps", bufs=1, space="PSUM") as pvps,
                    tc.tile_pool(name="dnps", bufs=1, space="PSUM") as dnps,
                ):
                    # ---- PE p-state warm-up: tiny dummy matmuls keep PE
                    # busy until the first input DMAs land (idle resets
                    # the ramp, so bridge the whole window) ----
                    wps = sps.tile([P, 512], F32, tag="s", name="wu")
                    for i in range(N_WARMUP):
                        nc.tensor.matmul(
                            wps[0:64, 0:64],
                            dummy_sb[:, :],
                            dummy_sb[:, :],
                            start=True,
                            stop=True,
                        )

                    dn_t = dnps.tile([P, 2, 8], F32, name="dn")
                    nc.vector.memset(dn_t[:, :, :], 0.0)

                    # engine-balance accumulators (ns)
                    eng_load = {"A": 0.0, "D": 0.0}

                    def pick_engine(act_cost, dve_cost):
                        if eng_load["A"] + act_cost <= eng_load["D"] + dve_cost:
                            eng_load["A"] += act_cost
                            return "A"
                        eng_load["D"] += dve_cost
                        return "D"

                    def emit_proj(w3, msl, j, moving_x, out_w, out_cb,
                                  nout=512):
                        """3-term fp8 DR projection into one PSUM group."""
                        ps = sps.tile([P, 512], F32, tag="s", name="pj")
                        wa, wb, wr = w3
                        first = True
                        for wt, xt in ((wa, x8_sb), (wb, xr_sb), (wr, x8_sb)):
                            for cp in range(CP):
                                if moving_x:
                                    lhs = wt[:, cp, :, msl]
                                    rhs = xt[:, j, cp, :, :]
                                else:
                                    lhs = xt[:, j, cp, :, msl]
                                    rhs = wt[:, cp, :, :]
                                nc.tensor.matmul(
                                    ps[:, 0:nout],
                                    lhs,
                                    rhs,
                                    start=first,
                                    stop=(wt is wr and cp == CP - 1),
                                    perf_mode=DR,
                                    skip_group_check=True,
                                )
                                first = False
                        out_cb(ps)

                    def do_qk(which, m, j):
                        w3 = tuple(w_sb[f"w{which}{v}"] for v in ("a", "b", "r"))

                        def evict(ps):
                            out_ap = (qT_tiles if which == "q" else kT_tiles)[m][
                                :, s512(j)
                            ]
                            e = pick_engine(570.0, 658.0)
                            if which == "q":
                                if e == "A":
                                    nc.scalar.activation(
                                        out=out_ap,
                                        in_=ps[:, :],
                                        func=mybir.ActivationFunctionType.Identity,
                                        bias=bq_sb[:, m : m + 1],
                                    )
                                else:
                                    nc.vector.tensor_scalar(
                                        out_ap,
                                        ps[:, :],
                                        1.0,
                                        bq_sb[:, m : m + 1],
                                        mybir.AluOpType.mult,
                                        mybir.AluOpType.add,
                                    )
                            else:
                                if e == "A":
                                    nc.scalar.activation(
                                        out=out_ap,
                                        in_=ps[:, :],
                                        func=mybir.ActivationFunctionType.Copy,
                                    )
                                else:
                                    nc.vector.tensor_copy(out_ap, ps[:, :])

                        emit_proj(w3, slice(m * P, (m + 1) * P), j, True, None,
                                  evict)

                    def do_v(l):
                        w3 = tuple(w_sb[f"wv{v}"] for v in ("a", "b", "r"))

                        def evict(ps):
                            e = pick_engine(463.0, 525.0)
                            dst = v_sb[:, l, :, :]
                            src = ps[:, 0:HO].rearrange("p (h d) -> p h d", d=HD)
                            if e == "A":
                                nc.scalar.activation(
                                    out=dst, in_=src,
                                    func=mybir.ActivationFunctionType.Copy,
                                )
                            else:
                                nc.vector.tensor_copy(dst, src)

                        emit_proj(w3, slice((l % 4) * P, (l % 4 + 1) * P),
                                  l // 4, False, None, evict, nout=HO)

                    def emit_scores_exp(hl, lk, q4):
                        pc, odd = hl // 2, hl % 2
                        r0 = odd * HD
                        s_t = sps.tile([P, 512], F32, tag="s", name="sc")
                        nc.tensor.matmul(
                            s_t[:, :],
                            kT_tiles[pc][r0 : r0 + HD, lk * P : (lk + 1) * P],
                            qT_tiles[pc][r0 : r0 + HD, s512(q4)],
                            start=True,
                            stop=True,
                        )
                        p_t = ppool.tile([P, 512], F16, tag="p", name="pt")
                        if pick_engine(ACT_EXP, DVE_EXP) == "A":
                            nc.scalar.activation(
                                out=p_t[:, :],
                                in_=s_t[:, :],
                                func=mybir.ActivationFunctionType.Exp,
                                bias=shift_sb[:, 0:1],
                                scale=ACT_SCALE,
                            )
                        else:
                            nc.vector.tensor_scalar(
                                p_t[:, :].bitcast(U16),
                                s_t[:, :],
                                SCH_A,
                                SCH_B,
                                mybir.AluOpType.mult,
                                mybir.AluOpType.add,
                            )
                        return p_t

                    def emit_pv(pv, hl, lk, q4, p_t, sweep):
                        first = lk == 0 and q4 % 2 == 0
                        last = lk == LC - 1
                        for jj in range(4):
                            qc8 = (q4 % 2) * 4 + jj
                            nc.tensor.matmul(
                                pv[:, qc8, :],
                                p_t[:, jj * P : (jj + 1) * P],
                                v_sb[:, lk, hl, :],
                                start=(first and jj == 0),
                                stop=last,
                                skip_group_check=True,
                            )
                            nc.tensor.matmul(
                                dn_t[:, hl % 2, qc8 : qc8 + 1],
                                p_t[:, jj * P : (jj + 1) * P],
                                ones_sb[:, :],
                                start=False,
                                stop=last,
                                skip_group_check=True,
                            )

                    def evict_ao(pv, hl, sweep, split=False):
                        """recip + batched scaled eviction (256*attn, fp16)."""
                        rstage = rpool.tile([P, 8], F32, tag="r")
                        nc.vector.reciprocal(rstage[:, :], dn_t[:, hl % 2, :])
                        eng_load["D"] += 135.0
                        if hl + 2 < HL or sweep == 0:
                            nc.vector.memset(dn_t[:, hl % 2, :], 0.0)
                        qc0 = sweep * 8
                        groups = ((0, 4), (4, 8)) if split else ((0, 8),)
                        for g0, g1 in groups:
                            rb = rstage[:, g0:g1, None].broadcast_to(
                                (P, g1 - g0, HD))
                            nc.vector.scalar_tensor_tensor(
                                ao_q[:, qc0 + g0 : qc0 + g1, hl, :],
                                pv[:, g0:g1, :],
                                0.0625,
                                rb,
                                mybir.AluOpType.mult,
                                mybir.AluOpType.mult,
                            )
                            eng_load["D"] += 658.0 / len(groups)
                            if split:
                                quad_transpose(2 + (g0 // 4))

                    def quad_transpose(g):
                        nc.sync.dma_start_transpose(
                            ao_t[:, 4 * g : 4 * g + 4, :, :],
                            ao_q[:, 4 * g : 4 * g + 4, :, :],
                        )

                    def do_outproj(m):
                        y_t = ypool.tile([P, D], F16, tag="yt")
                        for n0, nsz in ((0, 512), (512, 256)):
                            ps = sps.tile([P, 512], F32, tag="s", name="yp")
                            for c in range(MC):
                                nc.tensor.matmul(
                                    ps[:, 0:nsz],
                                    ao_t[:, m, c, :],
                                    wo_sb[:, c, n0 : n0 + nsz],
                                    start=(c == 0),
                                    stop=(c == MC - 1),
                                )
                            e = pick_engine(
                                (nsz + 172) * 0.8333, (nsz + 120) * 1.0417)
                            if e == "A":
                                nc.scalar.activation(
                                    out=y_t[:, n0 : n0 + nsz], in_=ps[:, 0:nsz],
                                    func=mybir.ActivationFunctionType.Copy,
                                )
                            else:
                                nc.vector.tensor_copy(
                                    y_t[:, n0 : n0 + nsz], ps[:, 0:nsz])
                        yeng = nc.sync if m % 2 == 0 else nc.scalar
                        yeng.dma_start(y[m * P : (m + 1) * P, :], y_t[:])

                    # ---------------- prelude ----------------
                    do_qk("q", 0, 0)
                    do_qk("k", 0, 0)
                    do_qk("q", 0, 1)
                    do_qk("k", 0, 1)
                    do_v(0)
                    do_v(1)
                    do_qk("q", 0, 2)
                    do_qk("k", 0, 2)
                    do_qk("q", 0, 3)
                    do_qk("k", 0, 3)

                    # jobs woven into the unit stream (unit = (sweep,hl,lk,q4h);
                    # 32 units per head per sweep)
                    prejobs = {}
                    for j in range(4):                     # qT/kT chunk 1
                        prejobs.setdefault(4 * j, []).append(
                            lambda j=j: do_qk("q", 1, j))
                        prejobs.setdefault(4 * j + 2, []).append(
                            lambda j=j: do_qk("k", 1, j))
                    for l in range(2, LC):                 # V chunks 2-15
                        prejobs.setdefault(l - 1, []).append(lambda l=l: do_v(l))
                    for j in range(4):                     # qT/kT chunk 2
                        prejobs.setdefault(32 + 4 * j, []).append(
                            lambda j=j: do_qk("q", 2, j))
                        prejobs.setdefault(32 + 4 * j + 2, []).append(
                            lambda j=j: do_qk("k", 2, j))
                    # sweep B: transposes of qc 0-7 (after the (s1,h0) pump
                    # evicts (s0,h5) at unit 192+PV_LAG+1), then outproj jobs
                    t0 = 192 + PV_LAG + 3
                    prejobs.setdefault(t0, []).append(lambda: quad_transpose(0))
                    prejobs.setdefault(t0 + 2, []).append(
                        lambda: quad_transpose(1))
                    for i in range(8):                     # outproj qc 0-7
                        prejobs.setdefault(t0 + 4 + 16 * i, []).append(
                            lambda i=i: do_outproj(i))

                    units = [
                        (sweep, hl, lk, q4h)
                        for sweep in range(2)
                        for hl in range(HL)
                        for lk in range(LC)
                        for q4h in range(2)
                    ]
                    pend = []
                    pv_state = {"tile": None, "key": None}

                    def pump_pv(entry):
                        hl, lk, q4, p_t, sweep = entry
                        key = (sweep, hl)
                        if pv_state["key"] != key:
                            if pv_state["tile"] is not None:
                                osweep, ohl = pv_state["key"]
                                evict_ao(pv_state["tile"], ohl, osweep)
                            pv_state["tile"] = pvps.tile(
                                [P, 8, HD], F32, tag="pv", name="pv")
                            pv_state["key"] = key
                        emit_pv(pv_state["tile"], hl, lk, q4, p_t, sweep)

                    for u, (sweep, hl, lk, q4h) in enumerate(units):
                        q4 = 2 * sweep + q4h
                        for job in prejobs.get(u, ()):
                            job()
                        p_t = emit_scores_exp(hl, lk, q4)
                        pend.append((hl, lk, q4, p_t, sweep))
                        while len(pend) > PV_LAG:
                            pump_pv(pend.pop(0))
                    while pend:
                        pump_pv(pend.pop(0))
                    # final head: split eviction, pipelining transpose+outproj
                    evict_ao(pv_state["tile"], HL - 1, 1, split=True)
                    for i in range(8, QC):
                        do_outproj(i)

    nc.compile()
    return nc


def _get_nc():
    global _NC
    if _NC is None:
        _NC = build()
    return _NC


E4NP = ml_dtypes.float8_e4m3


def _dr_rows_x(a):
    """[768, 2048] -> [128, 4, 3, 2, 512]: query-block-major DR layout;
    row (cp, t, p) holds input row cp*256 + t*128 + p."""
    return np.ascontiguousarray(
        a.reshape(CP, 2, P, 4, 512).transpose(2, 3, 0, 1, 4)
    )


def _w_tensors(W):
    """W [768, 384] fp32 -> (W256, W16, Wr) fp8, flattened [128, 3*2*384]
    with row (cp, t, p) holding input row cp*256 + t*128 + p."""
    Wa = (256.0 * W).astype(E4NP)
    Wb = (16.0 * W).astype(E4NP)
    Wr = (256.0 * W - Wa.astype(np.float32)).astype(E4NP)
    return tuple(
        np.ascontiguousarray(
            t.reshape(CP, 2, P, HO).transpose(2, 0, 1, 3).reshape(P, CP * 2 * HO)
        )
        for t in (Wa, Wb, Wr)
    )


def kernel(**inputs) -> np.ndarray:
    x = np.asarray(inputs["x"], dtype=np.float32)
    Wq = np.asarray(inputs["Wq"], dtype=np.float32)
    Wk = np.asarray(inputs["Wk"], dtype=np.float32)
    Wv = np.asarray(inputs["Wv"], dtype=np.float32)
    Wo = np.asarray(inputs["Wo"], dtype=np.float32)
    bq = np.asarray(inputs["bq"], dtype=np.float32)
    bv = np.asarray(inputs["bv"], dtype=np.float32)
    bo = np.asarray(inputs["bo"], dtype=np.float32)

    nc = _get_nc()

    in_maps = []
    for c in range(8):
        b, hg = c // 2, c % 2
        cs = slice(hg * HO, (hg + 1) * HO)
        xT = np.ascontiguousarray(x[b].T)               # [768, 2048]
        x8f = xT.astype(E4NP)
        xrf = (16.0 * (xT - x8f.astype(np.float32))).astype(E4NP)
        m = {"x8": _dr_rows_x(x8f), "xr": _dr_rows_x(xrf)}
        for t, W in (("q", Wq), ("k", Wk), ("v", Wv)):
            Ws = W[:, cs]
            for v, arr in zip(("a", "b", "r"), _w_tensors(Ws)):
                m[f"w{t}{v}"] = arr
        Wos = Wo[cs, :]                                  # [384, 768]
        m["wo"] = np.ascontiguousarray(
            Wos.reshape(MC, P, D).transpose(1, 0, 2).reshape(P, MC * D)
        ).astype(np.float16)
        m["bq"] = np.ascontiguousarray(256.0 * bq[cs])
        in_maps.append(m)

    res = run_bass_kernel_spmd(nc, in_maps, core_ids=list(range(8)))
    bias_full = bv @ Wo + bo
    out = np.empty((B, L, D), dtype=np.float32)
    for b in range(B):
        out[b] = (
            res.results[2 * b]["y"].astype(np.float32)
            + res.results[2 * b + 1]["y"].astype(np.float32)
        ) / 256.0 + bias_full
    return out
